# revision 23
# baseline (speedup 1.0000x reference)
"""Bass/Trainium2 kernel for nn_HMEClassification (hierarchical MoE), v2.

Data parallel across 8 cores (batch sharded). Per core xT [128d, 16384b],
processed in 512-wide b-tiles.

Per tile (TB=512):
  L1 (7 units x 4 h-blocks): weight-stationary bf16 MMs -> PSUM pairs
      [128, 1024] (2 banks), evacuated with fused bias+relu -> bf16 hsb.
      Experts (u=3..6) first so L2 can start early; gates (0..2) after.
  L2 experts: per k-chunk, expert pair MMs at tile_position (0,0)/(0,64)
      (concurrent col-groups), K-accumulated -> psE [128, 1024] (pair0|pair1).
      Exp evac with class-bias -> expc bf16.
  Gates (log-domain, single ACT table set 'natural_log_exp_and_others'):
      psG bank [68, 512]: G1 (0,0) rows 0-31 = (z0,-z0,0..); G2 (0,32) rows
      32-63 = (zA,-zA,zB,-zB,0..); ones-select S MMs (0,64) rows 64-67 =
      per-expert softmax denominators. In-place E=exp(-z-db) on rows 0-63,
      then ONE Ln over rows 0-68 with per-partition bias (+1 rows 0-63, +0
      rows 64-67) gives V = [-ln sigma terms | ln S] in SBUF f32.
  Coefficient broadcast via f32r matmul: psB[p,b] = sum_q M[q,p]*V[q,b]
      with M in {0,-1} -> delta = ln(root*gate/S) per 64-row block; cbc =
      exp(psB) (ACT); prod = expc*cbc (DVE/GpSimd); final sum over experts
      via stacked-identity MMs -> psO [64, 512]; copy -> DMA out.
"""

import json
import os
import tempfile

import ml_dtypes
import numpy as np

import concourse.bass as bass
import concourse.mybir as mybir
import concourse.tile as tile
from concourse import bacc
from concourse.bass_utils import run_bass_kernel_spmd


def _setup_act_tables():
    """Reorder act_info.json so 'natural_log_exp_and_others' is first: the
    table-set chooser picks the first set containing each function, so Exp
    and Ln then share one resident table set (no per-tile ACT_TABLE_LOAD
    thrash). Points both bacc (python) and walrus (--act-root-json) at the
    same reordered copy so set indices agree."""
    from neuronxcc.driver.Job import Job
    from neuronxcc.driver.jobs.support.FindActInfo import findActInfoFile
    src = findActInfoFile(Job.getPackageDir(), "gen3")
    src_dir = os.path.dirname(src)
    dst_dir = os.path.join(tempfile.gettempdir(), "pwp_nle_first")
    os.makedirs(dst_dir, exist_ok=True)
    for f in os.listdir(src_dir):
        link = os.path.join(dst_dir, f)
        if f != "act_info.json" and not os.path.exists(link):
            os.symlink(os.path.join(src_dir, f), link)
    info = json.load(open(src))
    sets = info["act_func_sets"]
    sets.sort(key=lambda e: e["name"] != "natural_log_exp_and_others")
    dst = os.path.join(dst_dir, "act_info.json")
    with open(dst, "w") as f:
        json.dump(info, f)
    os.environ["BASS_ACT_ROOT_JSON_PATH"] = dst

    import concourse.hw_specs as hw_specs
    tables = {
        ent["name"]: {
            mybir.ActivationFunctionType.from_pwp(v)
            for v in ent["act"].keys()
        }
        for ent in info["act_func_sets"]
    }
    bacc.get_activation_tables = lambda arch: tables
    hw_specs.get_activation_tables = lambda arch: tables


B, D, H, C = 131072, 128, 512, 64
NCORES = 8
TB = 512                # b-tile width
KH = H // 128           # 4 h-chunks of 128

F32 = mybir.dt.float32
F32R = mybir.dt.float32r
BF16 = mybir.dt.bfloat16

# ---- bf16 consts layout (columns in [128, NB] bf16 tensor) ----
W1_OFF = 0                       # 7 units * 512 = 3584
W2_OFF = W1_OFF + 7 * H          # 16 blocks (k*4+e) * 64 = 1024
G1_OFF = W2_OFF + 16 * 64        # 4 k * 32 cols (d0,-d0,0..)
G2A_OFF = G1_OFF + 4 * 32        # 4 k * 32 cols (dA,-dA,0..)
G2B_OFF = G2A_OFF + 4 * 32       # 4 k * 32 cols (0,0,dB,-dB,0..)
SA_OFF = G2B_OFF + 4 * 32        # 4 cols ones-select pair0
SB_OFF = SA_OFF + 4              # 4 cols ones-select pair1
ID_OFF = SB_OFF + 4              # 64 cols stacked identity
NB = ID_OFF + 64
# ---- fp32 consts layout ----
B1_OFF = 0                       # 28 cols (u*4+hb) L1 biases
EB_OFF = B1_OFF + 28             # 2 cols expert class biases (pair0, pair1)
GB_OFF = EB_OFF + 2              # 1 col gate-bias for E-exp (rows 0-35)
LB_OFF = GB_OFF + 1              # 1 col Ln bias (1.0 rows 0-63, 0.0 rows 64+)
NF = LB_OFF + 1
# ---- f32 M matrix [68, 256]: two 128-col blocks (pair0, pair1) ----


def _build_consts(gW1, gb1, gW2, gb2, eW1, eb1, eW2, eb2):
    cb = np.zeros((128, NB), dtype=np.float32)
    for u in range(3):
        cb[:, W1_OFF + u * H: W1_OFF + (u + 1) * H] = gW1[u]
    for e in range(4):
        cb[:, W1_OFF + (3 + e) * H: W1_OFF + (4 + e) * H] = eW1[e]
    for k in range(KH):
        for e in range(4):
            cb[:, W2_OFF + (k * 4 + e) * 64: W2_OFF + (k * 4 + e + 1) * 64] = \
                eW2[e, k * 128:(k + 1) * 128, :]
    v = gW2[:, :, 0] - gW2[:, :, 1]          # [3, 512]
    for k in range(KH):
        sl = slice(k * 128, (k + 1) * 128)
        cb[:, G1_OFF + k * 32 + 0] = v[0, sl]
        cb[:, G1_OFF + k * 32 + 1] = -v[0, sl]
        cb[:, G2A_OFF + k * 32 + 0] = v[1, sl]
        cb[:, G2A_OFF + k * 32 + 1] = -v[1, sl]
        cb[:, G2B_OFF + k * 32 + 0] = v[2, sl]
        cb[:, G2B_OFF + k * 32 + 1] = -v[2, sl]
    cb[0:64, SA_OFF + 0] = 1.0
    cb[64:128, SA_OFF + 1] = 1.0
    cb[0:64, SB_OFF + 2] = 1.0
    cb[64:128, SB_OFF + 3] = 1.0
    p = np.arange(128)
    cb[:, ID_OFF: ID_OFF + 64] = (p[:, None] % 64 == np.arange(64)[None, :])

    cf = np.zeros((128, NF), dtype=np.float32)
    b1 = np.concatenate([gb1, eb1], axis=0)  # [7, 512]
    for u in range(7):
        for hb in range(KH):
            cf[:, B1_OFF + u * 4 + hb] = b1[u, hb * 128:(hb + 1) * 128]
    cf[:64, EB_OFF + 0] = eb2[0]
    cf[64:, EB_OFF + 0] = eb2[1]
    cf[:64, EB_OFF + 1] = eb2[2]
    cf[64:, EB_OFF + 1] = eb2[3]
    db = gb2[:, 0] - gb2[:, 1]               # [3]
    # E-exp bias: E = exp(-(z + db)) = exp(-z + bias), bias rows:
    #   rows 0/1: -+db0 (z0/-z0), rows 32/33: -+dbA, rows 64/65: -+dbB
    gB = np.zeros(128, dtype=np.float32)
    gB[0], gB[1] = -db[0], db[0]
    gB[32], gB[33] = -db[1], db[1]
    gB[64], gB[65] = -db[2], db[2]
    cf[:, GB_OFF] = gB
    lb = np.zeros(128, dtype=np.float32)
    lb[0:96] = 1.0                           # ln(E + 1) rows; ln(S + 0) rows
    cf[:, LB_OFF] = lb

    # M [100, 256] f32: delta = sum_q M[q, p] * V[q, b]
    M = np.zeros((100, 256), dtype=np.float32)
    # pair0 (cols 0-127): A1 block rows {0, 32, 96}; A2 block {0, 33, 97}
    M[0, 0:64] = -1.0; M[32, 0:64] = -1.0; M[96, 0:64] = -1.0
    M[0, 64:128] = -1.0; M[33, 64:128] = -1.0; M[97, 64:128] = -1.0
    # pair1 (cols 128-255): B1 {1, 64, 98}; B2 {1, 65, 99}
    M[1, 128:192] = -1.0; M[64, 128:192] = -1.0; M[98, 128:192] = -1.0
    M[1, 192:256] = -1.0; M[65, 192:256] = -1.0; M[99, 192:256] = -1.0

    zero_b1 = not (np.any(gb1) or np.any(eb1))
    return cb.astype(ml_dtypes.bfloat16), cf, M, zero_b1


def _build_nc(n_tiles, bc, zero_b1):
    _setup_act_tables()
    nc = bacc.Bacc("TRN2", target_bir_lowering=False)
    xt = nc.dram_tensor("xt", [D, bc], BF16, kind="ExternalInput")
    cbd = nc.dram_tensor("cb", [128, NB], BF16, kind="ExternalInput")
    cfd = nc.dram_tensor("cf", [128, NF], F32, kind="ExternalInput")
    md = nc.dram_tensor("md", [100, 256], F32R, kind="ExternalInput")
    outT = nc.dram_tensor("outT", [C, bc], F32, kind="ExternalOutput")

    AF = mybir.ActivationFunctionType
    OP = mybir.AluOpType

    # expert units first so L2 can start early; gate units last
    U_ORDER = [3, 4, 5, 6, 0, 1, 2]

    with tile.TileContext(nc) as tc:
        with (
            tc.tile_pool(name="singles", bufs=1) as singles,
            tc.tile_pool(name="xp", bufs=3) as xp,
            tc.tile_pool(name="hp", bufs=3) as hp,
            tc.tile_pool(name="ep", bufs=2) as ep,
            tc.tile_pool(name="vp", bufs=2) as vp,
            tc.tile_pool(name="cp", bufs=2) as cp,
            tc.tile_pool(name="pp", bufs=2) as pp,
            tc.tile_pool(name="op", bufs=2) as op_pool,
            tc.tile_pool(name="psL", bufs=2, space="PSUM") as psLp,
            tc.tile_pool(name="psE", bufs=1, space="PSUM") as psEp,
            tc.tile_pool(name="psG", bufs=1, space="PSUM") as psGp,
            tc.tile_pool(name="psT", bufs=1, space="PSUM") as psTp,
        ):
            cs = singles.tile([128, NB], BF16)
            nc.sync.dma_start(out=cs, in_=cbd[:, :])
            cf = singles.tile([128, NF], F32)
            nc.sync.dma_start(out=cf, in_=cfd[:, :])
            mm_ = singles.tile([100, 256], F32R)
            nc.sync.dma_start(out=mm_, in_=md[:, :])

            def w1_ap(u, hb):
                a = W1_OFF + u * H + hb * 128
                return cs[:, a: a + 128]

            def w2_ap(k, e):
                a = W2_OFF + (k * 4 + e) * 64
                return cs[:, a: a + 64]

            id2 = cs[:, ID_OFF: ID_OFF + 64]
            # B-pair order: experts u=3..6 first (pairs 0-7), gates u=0..2
            # (pairs 8-13). Pair i covers (u, hb2) with hb2 in {0, 2}.
            BPAIRS = [(u, hb2) for u in U_ORDER for hb2 in (0, 2)]
            # ACT/DVE split of L1 evacs (6 ACT, 8 DVE), spread out
            ACT_EVAC = {1, 4, 6, 8, 11, 13}

            def issue_E(st):
                # bcast MMs + exp(delta) + prods for a finished tile
                st["prods"] = []
                for pair in range(2):
                    psB = psTp.tile([128, TB], F32, tag="tail")
                    nc.tensor.matmul(
                        psB, mm_[:, pair * 128:(pair + 1) * 128],
                        st["V"][:, :], start=True, stop=True)
                    cbc = cp.tile([128, TB], BF16, tag=f"c{pair}")
                    nc.scalar.activation(cbc, psB, AF.Exp)
                    prod = pp.tile([128, TB], BF16, tag=f"p{pair}")
                    nc.gpsimd.tensor_tensor(
                        prod, st["expc"][:, pair * TB:(pair + 1) * TB], cbc,
                        op=OP.mult)
                    st["prods"].append(prod)

            def issue_F(st):
                psO = psTp.tile([128, TB], F32, tag="tail")
                nc.tensor.matmul(psO[0:64, :], id2, st["prods"][0],
                                 start=True, stop=False)
                nc.tensor.matmul(psO[0:64, :], id2, st["prods"][1],
                                 start=False, stop=True)
                osb = op_pool.tile([64, TB], F32, tag="osb")
                nc.vector.tensor_copy(osb, psO[0:64, :])
                t0 = st["t"]
                nc.sync.dma_start(out=outT[:, t0 * TB:(t0 + 1) * TB], in_=osb)

            prev = None      # tile awaiting gates/S/E (t-1)
            prev2 = None     # tile awaiting F (t-2)

            def issue_G(st, k):
                # gates k-chunk: 3 concurrent col-groups
                psG, hap = st["psG"], st["h_ap"]
                nc.tensor.matmul(
                    psG[0:32, :],
                    cs[:, G1_OFF + k * 32: G1_OFF + k * 32 + 32],
                    hap(0, k), start=(k == 0), stop=(k == KH - 1),
                    tile_position=(0, 0))
                nc.tensor.matmul(
                    psG[32:64, :],
                    cs[:, G2A_OFF + k * 32: G2A_OFF + k * 32 + 32],
                    hap(1, k), start=(k == 0), stop=(k == KH - 1),
                    tile_position=(0, 32))
                nc.tensor.matmul(
                    psG[64:96, :],
                    cs[:, G2B_OFF + k * 32: G2B_OFF + k * 32 + 32],
                    hap(2, k), start=(k == 0), stop=(k == KH - 1),
                    tile_position=(0, 64))

            def issue_SVL(st):
                # S-MMs + E-exp + Ln for tile st
                psG, expc = st["psG"], st["expc"]
                nc.tensor.matmul(psG[96:100, :], cs[:, SA_OFF: SA_OFF + 4],
                                 expc[:, 0:TB], start=True, stop=False,
                                 tile_position=(0, 96))
                nc.tensor.matmul(psG[96:100, :], cs[:, SB_OFF: SB_OFF + 4],
                                 expc[:, TB:2 * TB], start=False, stop=True,
                                 tile_position=(0, 96))
                nc.scalar.activation(psG[0:96, :], psG[0:96, :], AF.Exp,
                                     bias=cf[0:96, GB_OFF: GB_OFF + 1],
                                     scale=-1.0)
                V = vp.tile([100, TB], F32R, tag="v")
                nc.scalar.activation(V, psG[0:100, :], AF.Ln,
                                     bias=cf[0:100, LB_OFF: LB_OFF + 1])
                st["V"] = V

            for t in range(n_tiles):
                xtile = xp.tile([D, TB], BF16, tag="x")
                nc.sync.dma_start(out=xtile, in_=xt[:, t * TB:(t + 1) * TB])
                hsb = {}

                def h_ap(u, k, hsb=hsb):
                    base = hsb[u, (k // 2) * 2]
                    j = k % 2
                    return base[:, j * TB:(j + 1) * TB]

                st = {"t": t, "h_ap": h_ap}
                psE = None
                for i, (u, hb2) in enumerate(BPAIRS):
                    ps = psLp.tile([128, 2 * TB], F32, tag="l1")
                    nc.tensor.matmul(ps[:, 0:TB], w1_ap(u, hb2), xtile,
                                     start=True, stop=True)
                    nc.tensor.matmul(ps[:, TB:2 * TB], w1_ap(u, hb2 + 1),
                                     xtile, start=True, stop=True)
                    h = hp.tile([128, 2 * TB], BF16, tag=f"h{u}_{hb2}")
                    if zero_b1:
                        if i in ACT_EVAC:
                            nc.scalar.activation(h, ps, AF.Relu)
                        else:
                            nc.vector.tensor_scalar(
                                h, ps, 0.0, None, op0=OP.max)
                    else:
                        for j in range(2):
                            bap = cf[:, B1_OFF + u * 4 + hb2 + j:
                                     B1_OFF + u * 4 + hb2 + j + 1]
                            hj = h[:, j * TB:(j + 1) * TB]
                            pj = ps[:, j * TB:(j + 1) * TB]
                            if i in ACT_EVAC:
                                nc.scalar.activation(hj, pj, AF.Relu,
                                                     bias=bap)
                            else:
                                nc.vector.tensor_scalar(
                                    hj, pj, bap, 0.0, op0=OP.add, op1=OP.max)
                    hsb[u, hb2] = h

                    # ---- software-pipelined interleave ----
                    if i <= 3 and prev is not None:
                        if i == 0:
                            psG_t = psGp.tile([100, TB], F32, tag="g")
                            prev["psG"] = psG_t
                        issue_G(prev, i)
                        if i == 3:
                            issue_SVL(prev)
                    if i == 5 and prev2 is not None:
                        issue_F(prev2)
                    if i in (7, 9, 11, 13):
                        if i == 7:
                            psE = psEp.tile([128, 2 * TB], F32, tag="e2")
                            st["psE"] = psE
                        p, kbase = {7: (0, 0), 9: (0, 2),
                                    11: (1, 0), 13: (1, 2)}[i]
                        for k in (kbase, kbase + 1):
                            sl = slice(p * TB, (p + 1) * TB)
                            nc.tensor.matmul(
                                psE[0:64, sl], w2_ap(k, 2 * p),
                                h_ap(3 + 2 * p, k), start=(k == 0),
                                stop=(k == KH - 1), tile_position=(0, 0))
                            nc.tensor.matmul(
                                psE[64:128, sl], w2_ap(k, 2 * p + 1),
                                h_ap(4 + 2 * p, k), start=(k == 0),
                                stop=(k == KH - 1), tile_position=(0, 64))
                expc = ep.tile([128, 2 * TB], BF16, tag="exp")
                st["expc"] = expc
                for pair in range(2):
                    sl = slice(pair * TB, (pair + 1) * TB)
                    nc.scalar.activation(
                        expc[:, sl], psE[:, sl], AF.Exp,
                        bias=cf[:, EB_OFF + pair: EB_OFF + pair + 1])
                if prev is not None:
                    issue_E(prev)
                    prev2 = prev

                prev = st

            # ---- epilogue: flush the last tiles ----
            psG_l = psGp.tile([100, TB], F32, tag="g")
            prev["psG"] = psG_l
            for k in range(KH):
                issue_G(prev, k)
            issue_SVL(prev)
            issue_F(prev2)
            issue_E(prev)
            issue_F(prev)

    nc.compile()
    return nc


def kernel(x, gW1, gb1, gW2, gb2, eW1, eb1, eW2, eb2, _trace=False):
    x = np.asarray(x, dtype=np.float32)
    cb, cf, M, zero_b1 = _build_consts(
        np.asarray(gW1, np.float32), np.asarray(gb1, np.float32),
        np.asarray(gW2, np.float32), np.asarray(gb2, np.float32),
        np.asarray(eW1, np.float32), np.asarray(eb1, np.float32),
        np.asarray(eW2, np.float32), np.asarray(eb2, np.float32))
    n_rows = x.shape[0]
    bc = n_rows // NCORES
    n_tiles = bc // TB
    assert bc * NCORES == n_rows and n_tiles * TB == bc

    nc = _build_nc(n_tiles, bc, zero_b1)

    xs = x.reshape(NCORES, bc, D)
    in_maps = [
        {"xt": np.ascontiguousarray(xs[c].T).astype(ml_dtypes.bfloat16),
         "cb": cb, "cf": cf, "md": M}
        for c in range(NCORES)
    ]
    res = run_bass_kernel_spmd(nc, in_maps, core_ids=list(range(NCORES)),
                               trace=_trace)
    out = np.concatenate([r["outT"].T for r in res.results], axis=0)
    kernel.last_results = res
    return np.ascontiguousarray(out.astype(np.float32))


# revision 24
# speedup vs baseline: 1.0038x; 1.0038x over previous
"""Bass/Trainium2 kernel for nn_HMEClassification (hierarchical MoE), v2.

Data parallel across 8 cores (batch sharded). Per core xT [128d, 16384b],
processed in 512-wide b-tiles.

Per tile (TB=512):
  L1 (7 units x 4 h-blocks): weight-stationary bf16 MMs -> PSUM pairs
      [128, 1024] (2 banks), evacuated with fused bias+relu -> bf16 hsb.
      Experts (u=3..6) first so L2 can start early; gates (0..2) after.
  L2 experts: per k-chunk, expert pair MMs at tile_position (0,0)/(0,64)
      (concurrent col-groups), K-accumulated -> psE [128, 1024] (pair0|pair1).
      Exp evac with class-bias -> expc bf16.
  Gates (log-domain, single ACT table set 'natural_log_exp_and_others'):
      psG bank [68, 512]: G1 (0,0) rows 0-31 = (z0,-z0,0..); G2 (0,32) rows
      32-63 = (zA,-zA,zB,-zB,0..); ones-select S MMs (0,64) rows 64-67 =
      per-expert softmax denominators. In-place E=exp(-z-db) on rows 0-63,
      then ONE Ln over rows 0-68 with per-partition bias (+1 rows 0-63, +0
      rows 64-67) gives V = [-ln sigma terms | ln S] in SBUF f32.
  Coefficient broadcast via f32r matmul: psB[p,b] = sum_q M[q,p]*V[q,b]
      with M in {0,-1} -> delta = ln(root*gate/S) per 64-row block; cbc =
      exp(psB) (ACT); prod = expc*cbc (DVE/GpSimd); final sum over experts
      via stacked-identity MMs -> psO [64, 512]; copy -> DMA out.
"""

import json
import os
import tempfile

import ml_dtypes
import numpy as np

import concourse.bass as bass
import concourse.mybir as mybir
import concourse.tile as tile
from concourse import bacc
from concourse.bass_utils import run_bass_kernel_spmd


def _setup_act_tables():
    """Reorder act_info.json so 'natural_log_exp_and_others' is first: the
    table-set chooser picks the first set containing each function, so Exp
    and Ln then share one resident table set (no per-tile ACT_TABLE_LOAD
    thrash). Points both bacc (python) and walrus (--act-root-json) at the
    same reordered copy so set indices agree."""
    from neuronxcc.driver.Job import Job
    from neuronxcc.driver.jobs.support.FindActInfo import findActInfoFile
    src = findActInfoFile(Job.getPackageDir(), "gen3")
    src_dir = os.path.dirname(src)
    dst_dir = os.path.join(tempfile.gettempdir(), "pwp_nle_first")
    os.makedirs(dst_dir, exist_ok=True)
    for f in os.listdir(src_dir):
        link = os.path.join(dst_dir, f)
        if f != "act_info.json" and not os.path.exists(link):
            os.symlink(os.path.join(src_dir, f), link)
    info = json.load(open(src))
    sets = info["act_func_sets"]
    sets.sort(key=lambda e: e["name"] != "natural_log_exp_and_others")
    dst = os.path.join(dst_dir, "act_info.json")
    with open(dst, "w") as f:
        json.dump(info, f)
    os.environ["BASS_ACT_ROOT_JSON_PATH"] = dst

    import concourse.hw_specs as hw_specs
    tables = {
        ent["name"]: {
            mybir.ActivationFunctionType.from_pwp(v)
            for v in ent["act"].keys()
        }
        for ent in info["act_func_sets"]
    }
    bacc.get_activation_tables = lambda arch: tables
    hw_specs.get_activation_tables = lambda arch: tables


B, D, H, C = 131072, 128, 512, 64
NCORES = 8
TB = 512                # b-tile width
KH = H // 128           # 4 h-chunks of 128

F32 = mybir.dt.float32
F32R = mybir.dt.float32r
BF16 = mybir.dt.bfloat16

# ---- bf16 consts layout (columns in [128, NB] bf16 tensor) ----
W1_OFF = 0                       # 7 units * 512 = 3584
W2_OFF = W1_OFF + 7 * H          # 16 blocks (k*4+e) * 64 = 1024
G1_OFF = W2_OFF + 16 * 64        # 4 k * 32 cols (d0,-d0,0..)
G2A_OFF = G1_OFF + 4 * 32        # 4 k * 32 cols (dA,-dA,0..)
G2B_OFF = G2A_OFF + 4 * 32       # 4 k * 32 cols (0,0,dB,-dB,0..)
SA_OFF = G2B_OFF + 4 * 32        # 4 cols ones-select pair0
SB_OFF = SA_OFF + 4              # 4 cols ones-select pair1
ID_OFF = SB_OFF + 4              # 64 cols stacked identity
NB = ID_OFF + 64
# ---- fp32 consts layout ----
B1_OFF = 0                       # 28 cols (u*4+hb) L1 biases
EB_OFF = B1_OFF + 28             # 2 cols expert class biases (pair0, pair1)
GB_OFF = EB_OFF + 2              # 1 col gate-bias for E-exp (rows 0-35)
LB_OFF = GB_OFF + 1              # 1 col Ln bias (1.0 rows 0-63, 0.0 rows 64+)
NF = LB_OFF + 1
# ---- f32 M matrix [68, 256]: two 128-col blocks (pair0, pair1) ----


def _build_consts(gW1, gb1, gW2, gb2, eW1, eb1, eW2, eb2):
    cb = np.zeros((128, NB), dtype=np.float32)
    for u in range(3):
        cb[:, W1_OFF + u * H: W1_OFF + (u + 1) * H] = gW1[u]
    for e in range(4):
        cb[:, W1_OFF + (3 + e) * H: W1_OFF + (4 + e) * H] = eW1[e]
    for k in range(KH):
        for e in range(4):
            cb[:, W2_OFF + (k * 4 + e) * 64: W2_OFF + (k * 4 + e + 1) * 64] = \
                eW2[e, k * 128:(k + 1) * 128, :]
    v = gW2[:, :, 0] - gW2[:, :, 1]          # [3, 512]
    for k in range(KH):
        sl = slice(k * 128, (k + 1) * 128)
        cb[:, G1_OFF + k * 32 + 0] = v[0, sl]
        cb[:, G1_OFF + k * 32 + 1] = -v[0, sl]
        cb[:, G2A_OFF + k * 32 + 0] = v[1, sl]
        cb[:, G2A_OFF + k * 32 + 1] = -v[1, sl]
        cb[:, G2B_OFF + k * 32 + 0] = v[2, sl]
        cb[:, G2B_OFF + k * 32 + 1] = -v[2, sl]
    cb[0:64, SA_OFF + 0] = 1.0
    cb[64:128, SA_OFF + 1] = 1.0
    cb[0:64, SB_OFF + 2] = 1.0
    cb[64:128, SB_OFF + 3] = 1.0
    p = np.arange(128)
    cb[:, ID_OFF: ID_OFF + 64] = (p[:, None] % 64 == np.arange(64)[None, :])

    cf = np.zeros((128, NF), dtype=np.float32)
    b1 = np.concatenate([gb1, eb1], axis=0)  # [7, 512]
    for u in range(7):
        for hb in range(KH):
            cf[:, B1_OFF + u * 4 + hb] = b1[u, hb * 128:(hb + 1) * 128]
    cf[:64, EB_OFF + 0] = eb2[0]
    cf[64:, EB_OFF + 0] = eb2[1]
    cf[:64, EB_OFF + 1] = eb2[2]
    cf[64:, EB_OFF + 1] = eb2[3]
    db = gb2[:, 0] - gb2[:, 1]               # [3]
    # E-exp bias: E = exp(-(z + db)) = exp(-z + bias), bias rows:
    #   rows 0/1: -+db0 (z0/-z0), rows 32/33: -+dbA, rows 64/65: -+dbB
    gB = np.zeros(128, dtype=np.float32)
    gB[0], gB[1] = -db[0], db[0]
    gB[32], gB[33] = -db[1], db[1]
    gB[64], gB[65] = -db[2], db[2]
    cf[:, GB_OFF] = gB
    lb = np.zeros(128, dtype=np.float32)
    lb[0:96] = 1.0                           # ln(E + 1) rows; ln(S + 0) rows
    cf[:, LB_OFF] = lb

    # M [100, 256] f32: delta = sum_q M[q, p] * V[q, b]
    M = np.zeros((100, 256), dtype=np.float32)
    # pair0 (cols 0-127): A1 block rows {0, 32, 96}; A2 block {0, 33, 97}
    M[0, 0:64] = -1.0; M[32, 0:64] = -1.0; M[96, 0:64] = -1.0
    M[0, 64:128] = -1.0; M[33, 64:128] = -1.0; M[97, 64:128] = -1.0
    # pair1 (cols 128-255): B1 {1, 64, 98}; B2 {1, 65, 99}
    M[1, 128:192] = -1.0; M[64, 128:192] = -1.0; M[98, 128:192] = -1.0
    M[1, 192:256] = -1.0; M[65, 192:256] = -1.0; M[99, 192:256] = -1.0

    zero_b1 = not (np.any(gb1) or np.any(eb1))
    zero_b2 = not np.any(eb2)
    return cb.astype(ml_dtypes.bfloat16), cf, M, zero_b1, zero_b2


def _build_nc(n_tiles, bc, zero_b1, zero_b2):
    _setup_act_tables()
    nc = bacc.Bacc("TRN2", target_bir_lowering=False)
    xt = nc.dram_tensor("xt", [D, bc], BF16, kind="ExternalInput")
    cbd = nc.dram_tensor("cb", [128, NB], BF16, kind="ExternalInput")
    cfd = nc.dram_tensor("cf", [128, NF], F32, kind="ExternalInput")
    md = nc.dram_tensor("md", [100, 256], F32R, kind="ExternalInput")
    outT = nc.dram_tensor("outT", [C, bc], F32, kind="ExternalOutput")

    AF = mybir.ActivationFunctionType
    OP = mybir.AluOpType

    # expert units first so L2 can start early; gate units last
    U_ORDER = [3, 4, 5, 6, 0, 1, 2]

    with tile.TileContext(nc) as tc:
        with (
            tc.tile_pool(name="singles", bufs=1) as singles,
            tc.tile_pool(name="xp", bufs=3) as xp,
            tc.tile_pool(name="hp", bufs=3) as hp,
            tc.tile_pool(name="ep", bufs=2) as ep,
            tc.tile_pool(name="vp", bufs=2) as vp,
            tc.tile_pool(name="cp", bufs=2) as cp,
            tc.tile_pool(name="pp", bufs=2) as pp,
            tc.tile_pool(name="op", bufs=2) as op_pool,
            tc.tile_pool(name="psL", bufs=2, space="PSUM") as psLp,
            tc.tile_pool(name="psE", bufs=1, space="PSUM") as psEp,
            tc.tile_pool(name="psG", bufs=1, space="PSUM") as psGp,
            tc.tile_pool(name="psT", bufs=1, space="PSUM") as psTp,
        ):
            cs = singles.tile([128, NB], BF16)
            nc.sync.dma_start(out=cs, in_=cbd[:, :])
            cf = singles.tile([128, NF], F32)
            nc.sync.dma_start(out=cf, in_=cfd[:, :])
            mm_ = singles.tile([100, 256], F32R)
            nc.sync.dma_start(out=mm_, in_=md[:, :])

            def w1_ap(u, hb):
                a = W1_OFF + u * H + hb * 128
                return cs[:, a: a + 128]

            def w2_ap(k, e):
                a = W2_OFF + (k * 4 + e) * 64
                return cs[:, a: a + 64]

            id2 = cs[:, ID_OFF: ID_OFF + 64]
            # B-pair order: experts u=3..6 first (pairs 0-7), gates u=0..2
            # (pairs 8-13). Pair i covers (u, hb2) with hb2 in {0, 2}.
            BPAIRS = [(u, hb2) for u in U_ORDER for hb2 in (0, 2)]
            # ACT/DVE split of L1 evacs (6 ACT, 8 DVE), spread out
            ACT_EVAC = {1, 4, 6, 8, 11, 13}

            def issue_E(st):
                # bcast MMs + exp(delta) + prods for a finished tile
                st["prods"] = []
                for pair in range(2):
                    psB = psTp.tile([128, TB], F32, tag="tail")
                    nc.tensor.matmul(
                        psB, mm_[:, pair * 128:(pair + 1) * 128],
                        st["V"][:, :], start=True, stop=True)
                    cbc = cp.tile([128, TB], BF16, tag=f"c{pair}")
                    nc.scalar.activation(cbc, psB, AF.Exp)
                    prod = pp.tile([128, TB], BF16, tag=f"p{pair}")
                    nc.gpsimd.tensor_tensor(
                        prod, st["expc"][:, pair * TB:(pair + 1) * TB], cbc,
                        op=OP.mult)
                    st["prods"].append(prod)

            def issue_F(st):
                psO = psTp.tile([128, TB], F32, tag="tail")
                nc.tensor.matmul(psO[0:64, :], id2, st["prods"][0],
                                 start=True, stop=False)
                nc.tensor.matmul(psO[0:64, :], id2, st["prods"][1],
                                 start=False, stop=True)
                osb = op_pool.tile([64, TB], F32, tag="osb")
                nc.vector.tensor_copy(osb, psO[0:64, :])
                t0 = st["t"]
                nc.sync.dma_start(out=outT[:, t0 * TB:(t0 + 1) * TB], in_=osb)

            prev = None      # tile awaiting gates/S/E (t-1)
            prev2 = None     # tile awaiting F (t-2)

            def issue_G(st, k):
                # gates k-chunk: 3 concurrent col-groups
                psG, hap = st["psG"], st["h_ap"]
                nc.tensor.matmul(
                    psG[0:32, :],
                    cs[:, G1_OFF + k * 32: G1_OFF + k * 32 + 32],
                    hap(0, k), start=(k == 0), stop=(k == KH - 1),
                    tile_position=(0, 0))
                nc.tensor.matmul(
                    psG[32:64, :],
                    cs[:, G2A_OFF + k * 32: G2A_OFF + k * 32 + 32],
                    hap(1, k), start=(k == 0), stop=(k == KH - 1),
                    tile_position=(0, 32))
                nc.tensor.matmul(
                    psG[64:96, :],
                    cs[:, G2B_OFF + k * 32: G2B_OFF + k * 32 + 32],
                    hap(2, k), start=(k == 0), stop=(k == KH - 1),
                    tile_position=(0, 64))

            def issue_SVL(st):
                # S-MMs + E-exp + Ln for tile st
                psG, expc = st["psG"], st["expc"]
                nc.tensor.matmul(psG[96:100, :], cs[:, SA_OFF: SA_OFF + 4],
                                 expc[:, 0:TB], start=True, stop=False,
                                 tile_position=(0, 96))
                nc.tensor.matmul(psG[96:100, :], cs[:, SB_OFF: SB_OFF + 4],
                                 expc[:, TB:2 * TB], start=False, stop=True,
                                 tile_position=(0, 96))
                nc.scalar.activation(psG[0:96, :], psG[0:96, :], AF.Exp,
                                     bias=cf[0:96, GB_OFF: GB_OFF + 1],
                                     scale=-1.0)
                V = vp.tile([100, TB], F32R, tag="v")
                nc.scalar.activation(V, psG[0:100, :], AF.Ln,
                                     bias=cf[0:100, LB_OFF: LB_OFF + 1])
                st["V"] = V

            for t in range(n_tiles):
                xtile = xp.tile([D, TB], BF16, tag="x")
                nc.sync.dma_start(out=xtile, in_=xt[:, t * TB:(t + 1) * TB])
                hsb = {}

                def h_ap(u, k, hsb=hsb):
                    base = hsb[u, (k // 2) * 2]
                    j = k % 2
                    return base[:, j * TB:(j + 1) * TB]

                st = {"t": t, "h_ap": h_ap}
                psE = None
                for i, (u, hb2) in enumerate(BPAIRS):
                    ps = psLp.tile([128, 2 * TB], F32, tag="l1")
                    nc.tensor.matmul(ps[:, 0:TB], w1_ap(u, hb2), xtile,
                                     start=True, stop=True)
                    nc.tensor.matmul(ps[:, TB:2 * TB], w1_ap(u, hb2 + 1),
                                     xtile, start=True, stop=True)
                    h = hp.tile([128, 2 * TB], BF16, tag=f"h{u}_{hb2}")
                    if zero_b1:
                        if i in ACT_EVAC:
                            nc.scalar.activation(h, ps, AF.Relu)
                        else:
                            nc.vector.tensor_scalar(
                                h, ps, 0.0, None, op0=OP.max)
                    else:
                        for j in range(2):
                            bap = cf[:, B1_OFF + u * 4 + hb2 + j:
                                     B1_OFF + u * 4 + hb2 + j + 1]
                            hj = h[:, j * TB:(j + 1) * TB]
                            pj = ps[:, j * TB:(j + 1) * TB]
                            if i in ACT_EVAC:
                                nc.scalar.activation(hj, pj, AF.Relu,
                                                     bias=bap)
                            else:
                                nc.vector.tensor_scalar(
                                    hj, pj, bap, 0.0, op0=OP.add, op1=OP.max)
                    hsb[u, hb2] = h

                    # ---- software-pipelined interleave ----
                    if i <= 3 and prev is not None:
                        if i == 0:
                            psG_t = psGp.tile([100, TB], F32, tag="g")
                            prev["psG"] = psG_t
                        issue_G(prev, i)
                        if i == 3:
                            issue_SVL(prev)
                    if i == 5 and prev2 is not None:
                        issue_F(prev2)
                    if i in (7, 9, 11, 13):
                        if i == 7:
                            psE = psEp.tile([128, 2 * TB], F32, tag="e2")
                            st["psE"] = psE
                        p, kbase = {7: (0, 0), 9: (0, 2),
                                    11: (1, 0), 13: (1, 2)}[i]
                        for k in (kbase, kbase + 1):
                            sl = slice(p * TB, (p + 1) * TB)
                            nc.tensor.matmul(
                                psE[0:64, sl], w2_ap(k, 2 * p),
                                h_ap(3 + 2 * p, k), start=(k == 0),
                                stop=(k == KH - 1), tile_position=(0, 0))
                            nc.tensor.matmul(
                                psE[64:128, sl], w2_ap(k, 2 * p + 1),
                                h_ap(4 + 2 * p, k), start=(k == 0),
                                stop=(k == KH - 1), tile_position=(0, 64))
                expc = ep.tile([128, 2 * TB], BF16, tag="exp")
                st["expc"] = expc
                if zero_b2:
                    nc.scalar.activation(expc, psE, AF.Exp)
                else:
                    for pair in range(2):
                        sl = slice(pair * TB, (pair + 1) * TB)
                        nc.scalar.activation(
                            expc[:, sl], psE[:, sl], AF.Exp,
                            bias=cf[:, EB_OFF + pair: EB_OFF + pair + 1])
                if prev is not None:
                    issue_E(prev)
                    prev2 = prev

                prev = st

            # ---- epilogue: flush the last tiles ----
            psG_l = psGp.tile([100, TB], F32, tag="g")
            prev["psG"] = psG_l
            for k in range(KH):
                issue_G(prev, k)
            issue_SVL(prev)
            issue_F(prev2)
            issue_E(prev)
            issue_F(prev)

    nc.compile()
    return nc


def kernel(x, gW1, gb1, gW2, gb2, eW1, eb1, eW2, eb2, _trace=False):
    x = np.asarray(x, dtype=np.float32)
    cb, cf, M, zero_b1, zero_b2 = _build_consts(
        np.asarray(gW1, np.float32), np.asarray(gb1, np.float32),
        np.asarray(gW2, np.float32), np.asarray(gb2, np.float32),
        np.asarray(eW1, np.float32), np.asarray(eb1, np.float32),
        np.asarray(eW2, np.float32), np.asarray(eb2, np.float32))
    n_rows = x.shape[0]
    bc = n_rows // NCORES
    n_tiles = bc // TB
    assert bc * NCORES == n_rows and n_tiles * TB == bc

    nc = _build_nc(n_tiles, bc, zero_b1, zero_b2)

    xs = x.reshape(NCORES, bc, D)
    in_maps = [
        {"xt": np.ascontiguousarray(xs[c].T).astype(ml_dtypes.bfloat16),
         "cb": cb, "cf": cf, "md": M}
        for c in range(NCORES)
    ]
    res = run_bass_kernel_spmd(nc, in_maps, core_ids=list(range(NCORES)),
                               trace=_trace)
    out = np.concatenate([r["outT"].T for r in res.results], axis=0)
    kernel.last_results = res
    return np.ascontiguousarray(out.astype(np.float32))
